# revision 1
# baseline (speedup 1.0000x reference)
"""GAT message-passing kernel for 8 Trainium2 NeuronCores.

Strategy (dst-sharded, per core):
  - Nodes are partitioned across 8 cores by destination id (12500 dst nodes
    per core); every edge (plus self-loops) is owned by the core owning its
    destination.
  - Phase 0: each core computes h = x @ W, a_src = <h, att_src>,
    a_dst = <h, att_dst> for its own nodes, packs [h(60) | a_src(4)] rows and
    AllGathers them into a full 100128-row table (each core contributes
    12516 rows: 12500 data + 16 zero rows used as pad targets).
  - Phase 1: per src-chunk k (4 chunks of 25032 table rows so chunk-local
    indices fit int16 for dma_gather), edges are laid out in a CSR slot grid
    [128 dst x D columns] per tile, degree-sorted per chunk so padding is
    small.  One dma_gather per group of tiles fetches packed rows per edge.
    Unnormalized attention w = exp(leaky_relu(a_src + a_dst)) (softmax
    normalization cancels, and fp32 holds exp of these logits safely), then
    per-dst row-sums produce partial numerator [60] and denominator [4].
  - Phase 2: the 4 chunk-partial outputs are re-gathered into natural dst
    order, summed, pad contributions to the denominator are removed
    analytically (pad rows are zero, so each pad adds exp(leaky(a_dst))),
    then out = elu(num/den + bias) @ lin_w + lin_b -> log_softmax.
"""
import sys

sys.path.insert(0, "/opt/trn_rl_repo")

import numpy as np

N, E = 100000, 1600000
IN_DIM, HEADS, HID, OUT_DIM = 128, 4, 15, 10
NEG_SLOPE = 0.2
NCORES = 8
NPC = N // NCORES              # 12500 dst nodes per core
T = (NPC + 127) // 128         # 98 tiles
NPCP = 128 * T                 # 12544 padded locals
SH = NPC + 16                  # 12516 shard rows contributed to allgather
CHSZ = N // 4                  # 25000 nodes per chunk (2 cores)
CHROWS = 2 * SH                # 25032 table rows per chunk window
NTAB = NCORES * SH             # 100128 table rows
ROW = 64                       # packed row: h(60) + a_src(4)
F = HEADS * HID                # 60
GROUP_COLS = 64                # max CSR columns per dma_gather


def _wrap_idx(flat):
    """int32 flat index list -> [128, n//16] int16 wrapped layout."""
    n = len(flat)
    assert n % 16 == 0
    w = flat.astype(np.int16).reshape(n // 16, 16).T.copy()
    return np.tile(w, (8, 1))


def _preprocess(src, dst):
    """Build per-core schedules. Returns (global schedule, per-core arrays)."""
    core_of = dst // NPC
    per_core = []
    for c in range(NCORES):
        m = core_of == c
        s_c = src[m].astype(np.int64)
        d_loc = (dst[m] - c * NPC).astype(np.int64)
        chunks = []
        for k in range(4):
            km = (s_c // CHSZ) == k
            sk = s_c[km]
            # chunk-local table row: 2*SH*k + (sk - 25000k) + 16*(second core)
            loc = sk - k * CHSZ
            tloc = np.where(loc < NPC, loc, loc + 16).astype(np.int32)
            dk = d_loc[km].astype(np.int32)
            counts = np.bincount(dk, minlength=NPCP).astype(np.int32)
            order = np.argsort(-counts, kind="stable").astype(np.int32)
            oi = np.empty(NPCP, np.int32)
            oi[order] = np.arange(NPCP, dtype=np.int32)
            D_ck = counts[order[0::128]]
            chunks.append(dict(tloc=tloc, dk=dk, counts=counts, order=order,
                               oi=oi, D_ck=D_ck))
        per_core.append(chunks)
    D = [np.max([per_core[c][k]["D_ck"] for c in range(NCORES)], axis=0)
         .astype(np.int32) for k in range(4)]
    return D, per_core


def _build_core_arrays(D, chunks):
    """Per-core gather index arrays and pad counts."""
    gidx_parts = []
    aidx_parts = []
    rgidx_parts = []
    P = np.zeros(NPCP, np.float32)
    for k in range(4):
        d = chunks[k]
        Dk = D[k]
        base = np.concatenate([[0], np.cumsum(128 * Dk.astype(np.int64))])
        NI = int(base[-1])
        # pad target: zero row 12500 of the chunk's first core block
        idx_flat = np.full(NI, NPC, np.int32)
        o = np.argsort(d["dk"], kind="stable")
        dks = d["dk"][o]
        tls = d["tloc"][o]
        starts = np.concatenate([[0], np.cumsum(d["counts"])]).astype(np.int64)
        rank = np.arange(len(dks), dtype=np.int64) - starts[dks]
        oi = d["oi"][dks]
        tau, pp = oi // 128, oi % 128
        linear = base[tau] + rank * 128 + pp
        idx_flat[linear] = tls
        gidx_parts.append(_wrap_idx(idx_flat))
        # a_dst gather: sub position (tau, p') -> local node id
        aidx_parts.append(_wrap_idx(d["order"].astype(np.int32)))
        # combine regather: final slot (p, t) i.e. local = 98p+t, output
        # linear i = t*128 + p  -> sub-out row = p'*T + tau
        loc = (np.arange(NPCP, dtype=np.int32).reshape(T, 128) % 128) * T \
            + np.arange(T, dtype=np.int32)[:, None]
        # loc[t, p] = 98*p + t ; sub row of local L: (oi[L]%128)*T + oi[L]//128
        oiL = d["oi"][loc.reshape(-1)]
        subrow = (oiL % 128) * T + oiL // 128
        rgidx_parts.append(_wrap_idx(subrow.astype(np.int32)))
        P += (Dk[d["oi"] // 128] - d["counts"]).astype(np.float32)
    gidx = np.concatenate(gidx_parts, axis=1)
    aidx = np.concatenate(aidx_parts, axis=1)
    rgidx = np.concatenate(rgidx_parts, axis=1)
    # P laid out [128, T] with [p, t] = P[98p + t]
    Parr = P.reshape(T, 128).T.copy() if False else \
        P[(np.arange(128)[:, None] * T + np.arange(T)[None, :])]
    return gidx, aidx, rgidx, Parr.astype(np.float32)


def _groups_of(Dk):
    """Split tiles into gather groups with <= GROUP_COLS columns."""
    groups = []
    cur = []
    cols = 0
    for tau in range(T):
        dcol = int(Dk[tau])
        if dcol == 0:
            # zero-degree tail tiles: no slots at all
            continue
        if cols + dcol > GROUP_COLS and cur:
            groups.append(cur)
            cur = []
            cols = 0
        cur.append(tau)
        cols += dcol
    if cur:
        groups.append(cur)
    return groups


_P1_SKIP_COMPUTE = False
_P1_SKIP_GA = False
_P1_SKIP_GROUPS = False


def _build_program(D, phases=3):
    import concourse.bass as bass
    import concourse.bacc as bacc
    import concourse.tile as tile
    from concourse import mybir
    from concourse.masks import make_identity

    fp32 = mybir.dt.float32
    i16 = mybir.dt.int16
    AL = mybir.AluOpType
    AF = mybir.ActivationFunctionType

    NI = [int((128 * D[k].astype(np.int64)).sum()) for k in range(4)]
    NItot = sum(NI)

    nc = bacc.Bacc("TRN2", target_bir_lowering=False, debug=False,
                   num_devices=NCORES)

    xT = nc.dram_tensor("xT", [128, NPCP], fp32, kind="ExternalInput").ap()
    w_in = nc.dram_tensor("w_in", [128, F], fp32, kind="ExternalInput").ap()
    asrc_in = nc.dram_tensor("asrc_in", [128, F], fp32, kind="ExternalInput").ap()
    adst_in = nc.dram_tensor("adst_in", [128, F], fp32, kind="ExternalInput").ap()
    bias_in = nc.dram_tensor("bias_in", [128, F], fp32, kind="ExternalInput").ap()
    linw_in = nc.dram_tensor("linw_in", [F, OUT_DIM], fp32, kind="ExternalInput").ap()
    linb_in = nc.dram_tensor("linb_in", [128, OUT_DIM], fp32, kind="ExternalInput").ap()
    gidx_in = nc.dram_tensor("gidx_in", [128, NItot // 16], i16, kind="ExternalInput").ap()
    aidx_in = nc.dram_tensor("aidx_in", [128, 4 * NPCP // 16], i16, kind="ExternalInput").ap()
    rgidx_in = nc.dram_tensor("rgidx_in", [128, 4 * NPCP // 16], i16, kind="ExternalInput").ap()
    p_in = nc.dram_tensor("p_in", [128, T], fp32, kind="ExternalInput").ap()
    out_t = nc.dram_tensor("out", [128, T, OUT_DIM], fp32, kind="ExternalOutput").ap()

    tshard = nc.dram_tensor("tshard", [NPCP, ROW], fp32)
    afat = nc.dram_tensor("afat", [NPCP, ROW], fp32)
    agout = nc.dram_tensor("agout", [NTAB, ROW], fp32, addr_space="Shared")
    table = nc.dram_tensor("table", [NTAB, ROW], fp32)
    subout = [nc.dram_tensor(f"subout{k}", [NPCP, ROW], fp32) for k in range(4)]

    with tile.TileContext(nc) as tc:
        from contextlib import ExitStack
        with ExitStack() as ctx:
            singles = ctx.enter_context(tc.tile_pool(name="singles", bufs=1))
            # --- constants ---
            w_sb = singles.tile([128, F], fp32)
            nc.sync.dma_start(out=w_sb[:], in_=w_in[:])
            asrc_sb = singles.tile([128, F], fp32)
            nc.sync.dma_start(out=asrc_sb[:], in_=asrc_in[:])
            adstv_sb = singles.tile([128, F], fp32)
            nc.sync.dma_start(out=adstv_sb[:], in_=adst_in[:])
            bias_sb = singles.tile([128, F], fp32)
            nc.sync.dma_start(out=bias_sb[:], in_=bias_in[:])
            linw_sb = singles.tile([F, OUT_DIM], fp32)
            nc.sync.dma_start(out=linw_sb[:], in_=linw_in[:])
            linb_sb = singles.tile([128, OUT_DIM], fp32)
            nc.sync.dma_start(out=linb_sb[:], in_=linb_in[:])
            p_sb = singles.tile([128, T], fp32)
            nc.sync.dma_start(out=p_sb[:], in_=p_in[:])
            aidx_sb = singles.tile([128, 4 * NPCP // 16], i16)
            nc.sync.dma_start(out=aidx_sb[:], in_=aidx_in[:])
            rgidx_sb = singles.tile([128, 4 * NPCP // 16], i16)
            nc.sync.dma_start(out=rgidx_sb[:], in_=rgidx_in[:])
            ident = singles.tile([128, 128], fp32)
            make_identity(nc, ident[:])

            adst_nat = singles.tile([128, T, HEADS], fp32)

            # ---------------- phase 0: table build ----------------
            with (
                tc.tile_pool(name="p0x", bufs=3) as p0x,
                tc.tile_pool(name="p0ps", bufs=2, space="PSUM") as p0ps,
                tc.tile_pool(name="p0t", bufs=3) as p0t,
                tc.tile_pool(name="p0stag", bufs=1) as p0stag,
            ):
                tstag = p0stag.tile([128, T, ROW], fp32)
                astag = p0stag.tile([128, T, ROW], fp32)
                nc.vector.memset(astag[:], 0.0)
                for t in range(T):
                    xt = p0x.tile([128, 128], fp32)
                    nc.sync.dma_start(out=xt[:], in_=xT[:, 128 * t:128 * (t + 1)])
                    hps = p0ps.tile([128, F], fp32, space="PSUM")
                    nc.tensor.matmul(out=hps[:], lhsT=xt[:], rhs=w_sb[:],
                                     start=True, stop=True)
                    nc.vector.tensor_copy(out=tstag[:, t, 0:F], in_=hps[:])
                    tmp = p0t.tile([128, F], fp32)
                    nc.vector.tensor_tensor(out=tmp[:], in0=tstag[:, t, 0:F],
                                            in1=asrc_sb[:], op=AL.mult)
                    nc.vector.tensor_reduce(
                        out=tstag[:, t, F:F + HEADS],
                        in_=tmp[:].rearrange("p (h c) -> p h c", h=HEADS),
                        axis=mybir.AxisListType.X, op=AL.add)
                    tmp2 = p0t.tile([128, F], fp32)
                    nc.vector.tensor_tensor(out=tmp2[:], in0=tstag[:, t, 0:F],
                                            in1=adstv_sb[:], op=AL.mult)
                    nc.vector.tensor_reduce(
                        out=adst_nat[:, t, :],
                        in_=tmp2[:].rearrange("p (h c) -> p h c", h=HEADS),
                        axis=mybir.AxisListType.X, op=AL.add)
                nc.vector.tensor_copy(out=astag[:, :, 0:HEADS], in_=adst_nat[:])
                nc.sync.dma_start(
                    out=tshard[:].rearrange("(p t) d -> p (t d)", p=128),
                    in_=tstag[:].rearrange("p t d -> p (t d)"))
                nc.sync.dma_start(
                    out=afat[:].rearrange("(p t) d -> p (t d)", p=128),
                    in_=astag[:].rearrange("p t d -> p (t d)"))
                nc.gpsimd.collective_compute(
                    "AllGather", AL.bypass,
                    replica_groups=[list(range(NCORES))],
                    ins=[tshard[0:SH, :]],
                    outs=[agout[:]],
                )
                nc.sync.dma_start(out=table[:], in_=agout[:])
                if phases < 1:
                    nc.sync.dma_start(out=out_t[:],
                                      in_=tstag[:, :, 0:OUT_DIM])

            # ---------------- phase 1: per-chunk CSR pipelines ----------------
            gcol_off = 0  # offset into gidx (columns of wrapped idx = 16 idxs each)
            if phases >= 1:
              DMAXG = max(int(D[k].max()) for k in range(4))
              NIW = max(NI[k] // 16 for k in range(4))
              with (
                tc.tile_pool(name="p1g", bufs=3) as p1g,
                tc.tile_pool(name="p1s", bufs=4) as p1s,
                tc.tile_pool(name="p1prod", bufs=3) as p1prod,
                tc.tile_pool(name="p1stag", bufs=1) as p1stag,
                tc.tile_pool(name="p1a", bufs=1) as p1a,
                tc.tile_pool(name="p1idx", bufs=2) as p1idx,
            ):
                for k in range(4):
                    Dk = D[k]
                    base = np.concatenate([[0], np.cumsum(128 * Dk.astype(np.int64))])
                    gidx_k = p1idx.tile([128, NIW], i16, tag="gidx")
                    nc.sync.dma_start(
                        out=gidx_k[:, 0:NI[k] // 16],
                        in_=gidx_in[:, gcol_off:gcol_off + NI[k] // 16])
                    gcol_off += NI[k] // 16
                    kcol = 0
                    sstag = p1stag.tile([128, T, ROW], fp32, tag="sstag")
                    nc.vector.memset(sstag[:], 0.0)
                    # a_dst per sub position
                    adst_sub = p1a.tile([128, T, HEADS], fp32, tag="adst_sub")
                    if not _P1_SKIP_GA:
                        ga = p1a.tile([128, T, ROW], fp32, tag="ga")
                        nc.gpsimd.dma_gather(
                            out_ap=ga[:], in_ap=afat[:],
                            idxs_ap=aidx_sb[:, k * (NPCP // 16):(k + 1) * (NPCP // 16)],
                            num_idxs=NPCP, num_idxs_reg=NPCP, elem_size=ROW, single_packet=False)
                        nc.vector.tensor_copy(out=adst_sub[:], in_=ga[:, :, 0:HEADS])
                    else:
                        nc.vector.memset(adst_sub[:], 0.0)
                    for group in ([] if _P1_SKIP_GROUPS else _groups_of(Dk)):
                        g_cols = int(sum(Dk[tau] for tau in group))
                        n_idx = 128 * g_cols
                        gt = p1g.tile([128, g_cols, ROW], fp32, tag="gt")
                        nc.gpsimd.dma_gather(
                            out_ap=gt[:],
                            in_ap=table[:][CHROWS * k:, :],
                            idxs_ap=gidx_k[:, kcol:kcol + n_idx // 16],
                            num_idxs=n_idx, num_idxs_reg=n_idx, elem_size=ROW, single_packet=False)
                        kcol += n_idx // 16
                        o = 0
                        for tau in (group if not _P1_SKIP_COMPUTE else []):
                            dd = int(Dk[tau])
                            gv = gt[:, o:o + dd, :]
                            o += dd
                            sv = p1s.tile([128, DMAXG, HEADS], fp32, tag="sv")
                            adst_b = bass.AP(
                                tensor=adst_sub.tensor, offset=adst_sub[:, tau, :].offset,
                                ap=[adst_sub.ap[0], [0, dd], [1, HEADS]])
                            nc.vector.tensor_tensor(out=sv[:, 0:dd, :],
                                                    in0=gv[:, :, F:F + HEADS],
                                                    in1=adst_b, op=AL.add)
                            ev = p1s.tile([128, DMAXG, HEADS], fp32, tag="ev")
                            nc.vector.tensor_scalar_mul(ev[:, 0:dd, :], sv[:, 0:dd, :], NEG_SLOPE)
                            nc.vector.tensor_tensor(out=ev[:, 0:dd, :], in0=sv[:, 0:dd, :],
                                                    in1=ev[:, 0:dd, :], op=AL.max)
                            wv = p1s.tile([128, DMAXG, HEADS], fp32, tag="wv")
                            nc.scalar.activation(out=wv[:, 0:dd, :], in_=ev[:, 0:dd, :],
                                                 func=AF.Exp)
                            wt = bass.AP(tensor=wv.tensor, offset=wv.offset,
                                         ap=[wv.ap[0], [1, HEADS], [HEADS, dd]])
                            nc.vector.tensor_reduce(out=sstag[:, tau, F:F + HEADS],
                                                    in_=wt, axis=mybir.AxisListType.X,
                                                    op=AL.add)
                            prod = p1prod.tile([128, DMAXG, F], fp32, tag="prod")
                            w_b = bass.AP(tensor=wv.tensor, offset=wv.offset,
                                          ap=[wv.ap[0], [HEADS, dd], [1, HEADS], [0, HID]])
                            nc.vector.tensor_tensor(out=prod[:, 0:dd, :],
                                                    in0=gv[:, :, 0:F],
                                                    in1=w_b, op=AL.mult)
                            pt = bass.AP(tensor=prod.tensor, offset=prod.offset,
                                         ap=[prod.ap[0], [1, F], [F, dd]])
                            nc.vector.tensor_reduce(out=sstag[:, tau, 0:F], in_=pt,
                                                    axis=mybir.AxisListType.X, op=AL.add)
                    nc.sync.dma_start(
                        out=subout[k][:].rearrange("(p t) d -> p (t d)", p=128),
                        in_=sstag[:].rearrange("p t d -> p (t d)"))
                if phases == 1:
                    nc.sync.dma_start(out=out_t[:], in_=sstag[:, :, 0:OUT_DIM])

            # ---------------- phase 2: combine ----------------
            if phases >= 2:
              with (
                tc.tile_pool(name="p2s", bufs=2) as p2s,
                tc.tile_pool(name="p2acc", bufs=1) as p2acc,
                tc.tile_pool(name="p2ps", bufs=2, space="PSUM") as p2ps,
                tc.tile_pool(name="p2t", bufs=4) as p2t,
            ):
                S = p2acc.tile([128, T, ROW], fp32)
                for k in range(4):
                    rg = p2s.tile([128, T, ROW], fp32, tag="rg")
                    nc.gpsimd.dma_gather(
                        out_ap=rg[:], in_ap=subout[k][:],
                        idxs_ap=rgidx_sb[:, k * (NPCP // 16):(k + 1) * (NPCP // 16)],
                        num_idxs=NPCP, num_idxs_reg=NPCP, elem_size=ROW, single_packet=False)
                    if k == 0:
                        nc.vector.tensor_copy(out=S[:], in_=rg[:])
                    else:
                        nc.vector.tensor_tensor(out=S[:], in0=S[:], in1=rg[:], op=AL.add)
                # adst_nat is currently [p, t] = natural local (98p+t)?  No:
                # adst_nat[:, t, :] holds local 98p+t at partition p -> matches
                # final slot layout (p, t) = local 98p+t.  Good.
                lt = p2acc.tile([128, T, HEADS], fp32)
                nc.vector.tensor_scalar_mul(lt[:], adst_nat[:], NEG_SLOPE)
                nc.vector.tensor_tensor(out=lt[:], in0=adst_nat[:], in1=lt[:], op=AL.max)
                ti = p2acc.tile([128, T, HEADS], fp32)
                nc.scalar.activation(out=ti[:], in_=lt[:], func=AF.Exp)
                p_b = bass.AP(tensor=p_sb.tensor, offset=p_sb.offset,
                              ap=[p_sb.ap[0], [1, T], [0, HEADS]])
                nc.vector.tensor_tensor(out=ti[:], in0=ti[:], in1=p_b, op=AL.mult)
                nc.vector.tensor_tensor(out=S[:, :, F:F + HEADS],
                                        in0=S[:, :, F:F + HEADS], in1=ti[:],
                                        op=AL.subtract)
                nc.vector.tensor_scalar_max(S[:, :, F:F + HEADS],
                                            S[:, :, F:F + HEADS], 1e-30)
                rec = p2acc.tile([128, T, HEADS], fp32)
                nc.vector.reciprocal(out=rec[:], in_=S[:, :, F:F + HEADS])
                y = p2acc.tile([128, T, F], fp32)
                rec_b = bass.AP(tensor=rec.tensor, offset=rec.offset,
                                ap=[rec.ap[0], [HEADS, T], [1, HEADS], [0, HID]])
                nc.vector.tensor_tensor(out=y[:], in0=S[:, :, 0:F], in1=rec_b,
                                        op=AL.mult)
                bias_b = bass.AP(tensor=bias_sb.tensor, offset=bias_sb.offset,
                                 ap=[bias_sb.ap[0], [0, T], [1, F]])
                nc.vector.tensor_tensor(out=y[:], in0=y[:], in1=bias_b, op=AL.add)
                # elu(y) = max(y,0) + exp(min(y,0)) - 1
                mn = p2acc.tile([128, T, F], fp32)
                nc.vector.tensor_scalar_min(mn[:], y[:], 0.0)
                nc.scalar.activation(out=mn[:], in_=mn[:], func=AF.Exp)
                nc.vector.tensor_scalar_max(y[:], y[:], 0.0)
                nc.vector.tensor_scalar_add(y[:], y[:], -1.0)
                nc.vector.tensor_tensor(out=y[:], in0=y[:], in1=mn[:], op=AL.add)
                ostag = p2acc.tile([128, T, OUT_DIM], fp32)
                for t in range(T):
                    zps = p2ps.tile([F, 128], fp32, space="PSUM", tag="zps")
                    nc.tensor.transpose(out=zps[:], in_=y[:, t, :], identity=ident[:])
                    zT = p2t.tile([F, 128], fp32, tag="zT")
                    nc.vector.tensor_copy(out=zT[:], in_=zps[:])
                    ops_ = p2ps.tile([128, OUT_DIM], fp32, space="PSUM", tag="ops")
                    nc.tensor.matmul(out=ops_[:], lhsT=zT[:], rhs=linw_sb[:],
                                     start=True, stop=True)
                    zt = p2t.tile([128, OUT_DIM], fp32, tag="zt")
                    nc.vector.tensor_tensor(out=zt[:], in0=ops_[:], in1=linb_sb[:],
                                            op=AL.add)
                    mx = p2t.tile([128, 1], fp32, tag="mx")
                    nc.vector.tensor_reduce(out=mx[:], in_=zt[:],
                                            axis=mybir.AxisListType.X, op=AL.max)
                    u = p2t.tile([128, OUT_DIM], fp32, tag="u")
                    nc.vector.tensor_scalar(out=u[:], in0=zt[:], scalar1=mx[:, 0:1],
                                            scalar2=None, op0=AL.subtract)
                    qe = p2t.tile([128, OUT_DIM], fp32, tag="qe")
                    nc.scalar.activation(out=qe[:], in_=u[:], func=AF.Exp)
                    ssum = p2t.tile([128, 1], fp32, tag="ssum")
                    nc.vector.tensor_reduce(out=ssum[:], in_=qe[:],
                                            axis=mybir.AxisListType.X, op=AL.add)
                    lg = p2t.tile([128, 1], fp32, tag="lg")
                    nc.scalar.activation(out=lg[:], in_=ssum[:], func=AF.Ln)
                    nc.vector.tensor_scalar(out=ostag[:, t, :], in0=u[:],
                                            scalar1=lg[:, 0:1], scalar2=None,
                                            op0=AL.subtract)
                nc.sync.dma_start(out=out_t[:], in_=ostag[:])

    nc.compile()
    return nc


_PROGRAM_CACHE = {}


LAST_EXEC_NS = None


def kernel(**inputs):
    import os
    from concourse.bass_utils import run_bass_kernel_spmd

    x = np.asarray(inputs["x"], dtype=np.float32)
    ei = np.asarray(inputs["edge_index"])
    W = np.asarray(inputs["W"], dtype=np.float32)
    att_src = np.asarray(inputs["att_src"], dtype=np.float32)
    att_dst = np.asarray(inputs["att_dst"], dtype=np.float32)
    bias = np.asarray(inputs["bias"], dtype=np.float32)
    lin_w = np.asarray(inputs["lin_w"], dtype=np.float32)
    lin_b = np.asarray(inputs["lin_b"], dtype=np.float32)

    src = np.concatenate([ei[0], np.arange(N)]).astype(np.int64)
    dst = np.concatenate([ei[1], np.arange(N)]).astype(np.int64)

    D, per_core = _preprocess(src, dst)

    key = tuple(int(v) for k in range(4) for v in D[k])
    if key not in _PROGRAM_CACHE:
        _PROGRAM_CACHE[key] = _build_program(D)
    nc = _PROGRAM_CACHE[key]

    # shared parameter arrays
    w_arr = W.reshape(IN_DIM, F)
    asrc_arr = np.tile(att_src.reshape(1, F), (128, 1)).astype(np.float32)
    adst_arr = np.tile(att_dst.reshape(1, F), (128, 1)).astype(np.float32)
    bias_arr = np.tile(bias.reshape(1, F), (128, 1)).astype(np.float32)
    linb_arr = np.tile(lin_b.reshape(1, OUT_DIM), (128, 1)).astype(np.float32)

    # sigma permutation: device slot (p, t) (row 128t+p of xT columns) holds
    # local node 98p+t
    p_grid, t_grid = np.meshgrid(np.arange(128), np.arange(T), indexing="ij")
    sigma = (p_grid * T + t_grid)  # [128, T] local id for slot (p, t)

    in_maps = []
    for c in range(NCORES):
        gidx, aidx, rgidx, Parr = _build_core_arrays(D, per_core[c])
        xs = np.zeros((NPCP, IN_DIM), np.float32)
        # column 128t+p of xT must hold x[c*NPC + 98p+t]
        loc = sigma.T.reshape(-1)  # [128*T] ordered by (t, p) -> column index
        valid = loc < NPC
        xs[valid] = x[c * NPC + loc[valid]]
        in_maps.append({
            "xT": np.ascontiguousarray(xs.T),
            "w_in": w_arr,
            "asrc_in": asrc_arr,
            "adst_in": adst_arr,
            "bias_in": bias_arr,
            "linw_in": lin_w,
            "linb_in": linb_arr,
            "gidx_in": gidx,
            "aidx_in": aidx,
            "rgidx_in": rgidx,
            "p_in": Parr,
        })

    trace = os.environ.get("KERNEL_TRACE") == "1"
    res = run_bass_kernel_spmd(nc, in_maps, list(range(NCORES)), trace=trace)
    global LAST_EXEC_NS
    LAST_EXEC_NS = res.exec_time_ns

    out = np.empty((N, OUT_DIM), np.float32)
    for c in range(NCORES):
        buf = res.results[c]["out"]  # [128, T, OUT_DIM], slot (p,t) = local 98p+t
        flat = buf.reshape(128 * T, OUT_DIM)  # row 98p+t ... wait: row T*p+t
        out[c * NPC:(c + 1) * NPC] = flat[:NPC]
    return out

